# revision 1
# baseline (speedup 1.0000x reference)
"""MoE feed-forward (top-2 routing, E=8 experts) on 8 TRN2 NeuronCores.

Strategy: 8-way HIDDEN-dim split with host-side routing/dispatch.
  Every core processes ALL experts' gathered tokens, but only a 1/8
  slice of the hidden dimension (4 of 32 ht tiles of 128). Per-core
  work is exactly sum(c_e)/8 token-equivalents -- perfectly balanced
  regardless of routing skew -- and every core runs the SAME static
  program (per-core variation is only in which weight rows/cols the
  host gathers into the same-shaped input arrays).

  Per expert e and token block b (tokens in [feature, token] layout):
    P1: h[4x128, b] = silu((w1 slice) @ x[b]) * ((w3 slice) @ x[b])
    P2: y_partial[D, b] = (w2 slice) @ h  (contraction over the 512
        local h rows), scaled per-token by the routing weight.
  The host sums the 8 per-core partial outputs (f32) and scatter-adds
  into [T, D].

  P1 of block k and P2 of block k-1 are interleaved on the PE at the
  accumulation-group level, so there are no phase-boundary stalls; the
  pipeline also runs across expert boundaries. Weights stream on the
  gpsimd queue (no compute rides it, so WAR waits on recycled weight
  tiles delay nothing but later weight arrivals).

Mixed precision: the lowest-combine-weight FP8_FRAC fraction of the
8192 (token, expert) assignments run the whole GLU in fp8-e4m3 with
MatmulPerfMode.DoubleRow (2 contraction tiles per instruction, ~1.4x
PE throughput); the rest run bf16. Error contribution of an
assignment scales with its combine weight, so spending the error
budget on the low-weight tail is cheap: measured 1.4e-2 overall at
FP8_FRAC=0.22 vs 3.7e-3 pure-bf16 (gate 2e-2). All fp8 scaling
factors are powers of two folded into the host-side weight/scale
arrays; the device applies them for free via the routing-weight row.
"""
import sys

if "/opt/trn_rl_repo" not in sys.path:
    sys.path.insert(0, "/opt/trn_rl_repo")

import numpy as np
import ml_dtypes

import concourse.bass as bass
import concourse.mybir as mybir
from concourse import bacc
from concourse.tile import TileContext
from concourse.bass_utils import run_bass_kernel_spmd

BF16 = ml_dtypes.bfloat16
FP8 = ml_dtypes.float8_e4m3  # IEEE e4m3 (max 240) == TRN FP8_EXP4
P = 128
D = 2048     # model dim
H = 4096     # hidden dim
E = 8        # experts
TOP_K = 2
DO = D // P          # 16 contraction tiles for P1
DO2 = DO // 2        # 8 DoubleRow contraction pairs
HL = H // P // E     # 4 local ht tiles per core
DT = D // P          # 16 output-row tiles for P2
SC = 16.0            # fp8 weight scale (power of two; folded into csx)
FP8_FRAC = 0.0       # fp8 disabled: extra weight-copy DMA outweighs PE gain
B16 = 448            # max bf16 token block
B8 = 512             # max fp8 token block


def _route(x, router_w):
    """Top-2 expert selection + renormalized weights (float64 host math)."""
    logits = x.astype(np.float64) @ router_w.astype(np.float64).T
    m = logits.max(axis=1, keepdims=True)
    p = np.exp(logits - m)
    p /= p.sum(axis=1, keepdims=True)
    sel = np.argsort(-p, axis=1, kind="stable")[:, :TOP_K]
    rw = np.take_along_axis(p, sel, axis=1)
    rw /= rw.sum(axis=1, keepdims=True)
    return sel, rw.astype(np.float32)


def _seg_blocks(L, maxb, first_small=False, last_small=False):
    """Split a 16-multiple segment into 16-multiple blocks <= maxb.
    first_small carves a 256 head (shrinks the startup DMA wall);
    last_small carves a 160 tail (shrinks the final P2 drain)."""
    if L == 0:
        return []
    head = tail = 0
    if first_small and L > 256:
        head, L = 256, L - 256
    if last_small and L > 256:
        tail, L = 96, L - 96
    sizes = []
    if L:
        n = -(-L // maxb)
        base = (L // n) // 16 * 16
        k = (L - base * n) // 16
        sizes = [base + 16] * k + [base] * (n - k)
    if head:
        sizes = [head] + sizes
    if tail:
        sizes = sizes + [tail]
    return sizes


def _plan(counts16, counts8):
    """Block plan: list of (expert, kind, seg_off, tn). kind: 16 or 8.
    Within an expert: bf16 blocks first, fp8 blocks last (cheaper P2
    drain tail, and the bf16 head gives the weight stream time)."""
    blocks = []
    off16 = off8 = 0
    for e in range(E):
        t = 0
        for tn in _seg_blocks(
            counts16[e], B16, last_small=(e == E - 1 and counts8[e] == 0)
        ):
            blocks.append((e, 16, off16 + t, tn))
            t += tn
        off16 += counts16[e]
        t = 0
        for tn in _seg_blocks(counts8[e], B8, last_small=(e == E - 1)):
            blocks.append((e, 8, off8 + t, tn))
            t += tn
        off8 += counts8[e]
    return blocks, max(off16, 16), max(off8, 16)


def _build(counts16, counts8):
    """Bass program: per-core 1/8-H slice of all experts' GLU MLPs."""
    f32 = mybir.dt.float32
    bf16 = mybir.dt.bfloat16
    fp8 = mybir.dt.float8e4
    Silu = mybir.ActivationFunctionType.Silu
    DR = mybir.MatmulPerfMode.DoubleRow

    blocks, S16, S8 = _plan(counts16, counts8)

    nc = bacc.Bacc(None, target_bir_lowering=False)
    x16d = nc.dram_tensor("x16d", [P, DO, S16], bf16, kind="ExternalInput")
    x8d = nc.dram_tensor("x8d", [P, DO2, 2, S8], fp8, kind="ExternalInput")
    w1d16 = nc.dram_tensor("w1d16", [P, E, HL, DO, P], bf16, kind="ExternalInput")
    w3d16 = nc.dram_tensor("w3d16", [P, E, HL, DO, P], bf16, kind="ExternalInput")
    w1d8 = nc.dram_tensor("w1d8", [P, E, HL, DO2, 2, P], fp8, kind="ExternalInput")
    w3d8 = nc.dram_tensor("w3d8", [P, E, HL, DO2, 2, P], fp8, kind="ExternalInput")
    w2d16 = nc.dram_tensor("w2d16", [P, E, 4, 4, HL, P], bf16, kind="ExternalInput")
    w2d8 = nc.dram_tensor("w2d8", [P, E, 2, DT // 2, 2, 2, P], fp8, kind="ExternalInput")
    cs16d = nc.dram_tensor("cs16d", [S16], f32, kind="ExternalInput")
    cs8d = nc.dram_tensor("cs8d", [S8], f32, kind="ExternalInput")
    o16d = nc.dram_tensor("o16d", [D, S16], bf16, kind="ExternalOutput")
    o8d = nc.dram_tensor("o8d", [D, S8], bf16, kind="ExternalOutput")
    o16v = o16d.rearrange("(dt p) c -> p dt c", p=P)
    o8v = o8d.rearrange("(dt p) c -> p dt c", p=P)

    def bcast(ap):
        return bass.AP(tensor=ap.tensor, offset=ap.offset, ap=[[0, P], *ap.ap])

    with TileContext(nc) as tc:
        with (
            tc.tile_pool(name="x16p", bufs=4) as x16p,
            tc.tile_pool(name="x8p", bufs=4) as x8p,
            tc.tile_pool(name="h16p", bufs=2) as h16p,
            tc.tile_pool(name="h8p", bufs=2) as h8p,
            tc.tile_pool(name="w1p16", bufs=8) as w1p16,
            tc.tile_pool(name="w3p16", bufs=8) as w3p16,
            tc.tile_pool(name="w1p8", bufs=6) as w1p8,
            tc.tile_pool(name="w3p8", bufs=6) as w3p8,
            tc.tile_pool(name="w2p16", bufs=8) as w2p16,
            tc.tile_pool(name="w2p8", bufs=2) as w2p8,
            tc.tile_pool(name="stp", bufs=2) as stp,
            tc.tile_pool(name="yp", bufs=2) as yp,
            tc.tile_pool(name="csp", bufs=4) as csp,
            tc.tile_pool(name="ps13", bufs=2, space="PSUM") as ps13,
            tc.tile_pool(name="ps2", bufs=4, space="PSUM") as ps2,
        ):
            has16 = [counts16[e] > 0 for e in range(E)]
            has8 = [counts8[e] > 0 for e in range(E)]

            w1t16 = [[None] * HL for _ in range(E)]
            w3t16 = [[None] * HL for _ in range(E)]
            w1t8 = [[None] * HL for _ in range(E)]
            w3t8 = [[None] * HL for _ in range(E)]
            w2t16 = [[None] * 4 for _ in range(E)]
            w2t8 = [[None] * 2 for _ in range(E)]

            def wload_list(e):
                """Closures emitting expert e's weight DMAs, in need order:
                bf16 w1/w3 (first P1 blocks), bf16 w2 (first P2, one block
                later), then the fp8 copies (used by the trailing fp8
                segment)."""
                ops = []
                if has16[e]:
                    for hl in range(HL):
                        def f(e=e, hl=hl):
                            w1t16[e][hl] = w1p16.tile([P, DO, P], bf16, name="w1t16")
                            nc.gpsimd.dma_start(out=w1t16[e][hl][:], in_=w1d16[:, e, hl])
                            w3t16[e][hl] = w3p16.tile([P, DO, P], bf16, name="w3t16")
                            nc.gpsimd.dma_start(out=w3t16[e][hl][:], in_=w3d16[:, e, hl])
                        ops.append(f)
                    for g in range(4):
                        def f(e=e, g=g, q=None):
                            w2t16[e][g] = w2p16.tile([P, 4, HL, P], bf16, name="w2t16")
                            (q or nc.gpsimd).dma_start(
                                out=w2t16[e][g][:], in_=w2d16[:, e, g]
                            )
                        ops.append(f)
                if has8[e]:
                    for hl in range(HL):
                        def f(e=e, hl=hl):
                            w1t8[e][hl] = w1p8.tile([P, DO2, 2, P], fp8, name="w1t8")
                            nc.gpsimd.dma_start(out=w1t8[e][hl][:], in_=w1d8[:, e, hl])
                            w3t8[e][hl] = w3p8.tile([P, DO2, 2, P], fp8, name="w3t8")
                            nc.gpsimd.dma_start(out=w3t8[e][hl][:], in_=w3d8[:, e, hl])
                        ops.append(f)
                    for i in range(2):
                        def f(e=e, i=i):
                            w2t8[e][i] = w2p8.tile(
                                [P, DT // 2, 2, 2, P], fp8, name="w2t8"
                            )
                            nc.gpsimd.dma_start(out=w2t8[e][i][:], in_=w2d8[:, e, i])
                        ops.append(f)
                return ops

            def load_block(blk, xt, cst, first=False):
                """x (2 big chunks on scalar+sync) + combine-weight row.
                Few, large DMAs: each dma_start costs ~1.5us of queue
                serialization, so the critical path wants fat transfers."""
                e, kind, o, tn = blk
                nc.sync.dma_start(out=cst[:, :tn], in_=bcast(
                    (cs16d if kind == 16 else cs8d)[o : o + tn]))
                if kind == 16:
                    h = DO // 2
                    nc.sync.dma_start(
                        out=xt[:, :h, :tn], in_=x16d[:, :h, o : o + tn])
                    nc.gpsimd.dma_start(
                        out=xt[:, h:, :tn], in_=x16d[:, h:, o : o + tn])
                else:
                    h = DO2 // 2
                    nc.sync.dma_start(
                        out=xt[:, :h, :, :tn], in_=x8d[:, :h, :, o : o + tn])
                    nc.gpsimd.dma_start(
                        out=xt[:, h:, :, :tn], in_=x8d[:, h:, :, o : o + tn])

            def new_xt(kind):
                if kind == 16:
                    return x16p.tile([P, DO, B16], bf16, name="xt16")
                return x8p.tile([P, DO2, 2, B8], fp8, name="xt8")

            def p1_group(blk, xt, ht, hl):
                e, kind, o, tn = blk
                pg = ps13.tile([P, 512], f32, name="pg")[:, :tn]
                pu = ps13.tile([P, 512], f32, name="pu")[:, :tn]
                if kind == 16:
                    for dk in range(DO):
                        nc.tensor.matmul(
                            pg, w1t16[e][hl][:, dk], xt[:, dk, :tn],
                            start=(dk == 0), stop=(dk == DO - 1),
                        )
                        nc.tensor.matmul(
                            pu, w3t16[e][hl][:, dk], xt[:, dk, :tn],
                            start=(dk == 0), stop=(dk == DO - 1),
                        )
                else:
                    for dk in range(DO2):
                        nc.tensor.matmul(
                            pg, w1t8[e][hl][:, dk], xt[:, dk, :, :tn],
                            start=(dk == 0), stop=(dk == DO2 - 1), perf_mode=DR,
                        )
                        nc.tensor.matmul(
                            pu, w3t8[e][hl][:, dk], xt[:, dk, :, :tn],
                            start=(dk == 0), stop=(dk == DO2 - 1), perf_mode=DR,
                        )
                st = stp.tile([P, 512], f32, name="st")[:, :tn]
                nc.scalar.activation(st, pg, Silu)
                if kind == 16:
                    nc.vector.tensor_mul(ht[:, hl, :tn], st, pu)
                else:
                    nc.vector.tensor_mul(ht[:, hl // 2, hl % 2, :tn], st, pu)

            def p2_group(blk, ht, cst, ysb, dt):
                e, kind, o, tn = blk
                py = ps2.tile([P, 512], f32, name="py")[:, :tn]
                if kind == 16:
                    for hl in range(HL):
                        nc.tensor.matmul(
                            py, w2t16[e][dt // 4][:, dt % 4, hl], ht[:, hl, :tn],
                            start=(hl == 0), stop=(hl == HL - 1),
                        )
                else:
                    for hp in range(2):
                        nc.tensor.matmul(
                            py, w2t8[e][dt // 8][:, dt % 8, hp], ht[:, hp, :, :tn],
                            start=(hp == 0), stop=(hp == 1), perf_mode=DR,
                        )
                ysb = yp.tile([P, 512], bf16, name="ysb")[:, :tn]
                nc.vector.tensor_mul(ysb, py, cst[:, :tn])
                ov = o16v if kind == 16 else o8v
                # final expert: split out-DMAs with the (by now idle) weight
                # queue so the drain tail isn't serialized on one queue
                if e == E - 1:
                    q = (nc.sync, nc.gpsimd, nc.scalar)[dt % 3]
                else:
                    q = nc.sync
                q.dma_start(out=ov[:, dt, o : o + tn], in_=ysb)

            # ---- emission: expert-0 P1 weights, 2-block x lead, pipeline ----
            w0 = wload_list(0)
            n0 = (HL + 4) if has16[0] else 0  # bf16 w1/w3 pairs + w2 tiles
            bq = []  # (blk, xt, cst) loaded, awaiting compute

            def preload(j):
                blk = blocks[j]
                xt = new_xt(blk[1])
                cst = csp.tile([P, 512], f32, name="cst")
                load_block(blk, xt, cst)
                bq.append((blk, xt, cst))

            if has16[0]:
                # interleave by need-time: w13 hl0 | x b0 | w13 hl1 | x b1 |
                # w13 hl2,hl3 on gpsimd; w2 on the (still idle) sync queue
                w0[0]()
                preload(0)
                w0[1]()
                preload(1)
                w0[2]()
                w0[3]()
                for f in w0[HL : HL + 4]:
                    f(q=nc.sync)
            else:
                for f in w0[:n0]:
                    f()
                preload(0)
                preload(1)
            pending = w0[n0:] + wload_list(1)
            next_loaded = 1

            prev = None  # (blk, ht, cst) awaiting its P2
            for k, blk in enumerate(blocks):
                e, kind, o, tn = blk
                if k + 2 < len(blocks):
                    preload(k + 2)
                if e + 1 > next_loaded and e + 1 < E:
                    pending += wload_list(e + 1)
                    next_loaded = e + 1
                _, xt, cst = bq.pop(0)
                ht = (
                    h16p.tile([P, HL, B16], bf16, name="ht16")
                    if kind == 16
                    else h8p.tile([P, 2, 2, B8], fp8, name="ht8")
                )
                for hl in range(HL):
                    p1_group(blk, xt, ht, hl)
                    if prev is not None:
                        for dt in range(4 * hl, 4 * hl + 4):
                            p2_group(*prev, dt)
                    for f in pending[:3]:
                        f()
                    pending = pending[3:]
                prev = (blk, ht, cst, yp.tile([P, DT, 512], bf16, name="ysb"))
            for f in pending:
                f()
            for dt in range(DT):
                p2_group(*prev, dt)

    nc.compile()
    return nc, blocks, S16, S8


_cache = {}


def _get_program(counts16, counts8):
    key = (tuple(counts16), tuple(counts8))
    if key not in _cache:
        _cache[key] = _build(counts16, counts8)
    return _cache[key]


def _prep_weights(w1, w3, w2):
    """Per-core weight arrays in device layouts (see _build docstring)."""
    per_core = []
    w1_8 = np.clip(w1 * SC, -240, 240).astype(FP8)
    w3_8 = np.clip(w3 * SC, -240, 240).astype(FP8)
    w2_8 = np.clip(w2 * SC, -240, 240).astype(FP8)
    w1_16 = w1.astype(BF16)
    w3_16 = w3.astype(BF16)
    w2_16 = w2.astype(BF16)
    for i in range(E):
        r0 = 512 * i
        W116 = np.empty((P, E, HL, DO, P), BF16)
        W316 = np.empty((P, E, HL, DO, P), BF16)
        W18 = np.empty((P, E, HL, DO2, 2, P), FP8)
        W38 = np.empty((P, E, HL, DO2, 2, P), FP8)
        W216 = np.empty((P, E, 4, 4, HL, P), BF16)
        W28 = np.empty((P, E, 2, DT // 2, 2, 2, P), FP8)
        for e in range(E):
            W116[:, e] = w1_16[e, r0 : r0 + 512].reshape(HL, P, DO, P).transpose(3, 0, 2, 1)
            W316[:, e] = w3_16[e, r0 : r0 + 512].reshape(HL, P, DO, P).transpose(3, 0, 2, 1)
            W18[:, e] = (
                w1_8[e, r0 : r0 + 512].reshape(HL, P, DO2, 2, P).transpose(4, 0, 2, 3, 1)
            )
            W38[:, e] = (
                w3_8[e, r0 : r0 + 512].reshape(HL, P, DO2, 2, P).transpose(4, 0, 2, 3, 1)
            )
            W216[:, e] = (
                w2_16[e, :, r0 : r0 + 512].reshape(4, 4, P, HL, P).transpose(4, 0, 1, 3, 2)
            )
            W28[:, e] = (
                w2_8[e, :, r0 : r0 + 512]
                .reshape(2, DT // 2, P, 2, 2, P)
                .transpose(5, 0, 1, 3, 4, 2)
            )
        per_core.append(
            dict(w1d16=W116, w3d16=W316, w1d8=W18, w3d8=W38, w2d16=W216, w2d8=W28)
        )
    return per_core


_wcache = {"key": None, "val": None}


def kernel(x, router_w, w1, w3, w2, _trace=False):
    T = x.shape[0]
    x = np.asarray(x, np.float32)
    router_w = np.asarray(router_w, np.float32)
    w1 = np.asarray(w1, np.float32)
    w3 = np.asarray(w3, np.float32)
    w2 = np.asarray(w2, np.float32)
    assert x.shape[1] == D and router_w.shape == (E, D)
    assert w1.shape == w3.shape == (E, H, D) and w2.shape == (E, D, H)

    sel, rw = _route(x, router_w)

    # per-expert token lists, sorted by combine weight ascending
    toks, cws = [], []
    for e in range(E):
        mask = sel == e
        tok = np.nonzero(mask.any(axis=1))[0]
        cw = np.where(mask[tok, 0], rw[tok, 0], rw[tok, 1])
        o = np.argsort(cw, kind="stable")
        toks.append(tok[o])
        cws.append(cw[o])
    allw = np.sort(np.concatenate(cws))
    n8_target = int(FP8_FRAC * len(allw))
    thr = allw[n8_target - 1] if n8_target > 0 else -1.0

    counts16, counts8, k8s = [], [], []
    for e in range(E):
        k8 = int(np.searchsorted(cws[e], thr, side="right"))
        k8s.append(k8)
        counts8.append(-(-k8 // 16) * 16 if k8 else 0)
        n16 = len(toks[e]) - k8
        counts16.append(-(-n16 // 16) * 16 if n16 else 0)

    nc, blocks, S16, S8 = _get_program(counts16, counts8)

    # ---- host-side gathers into device layouts ----
    xg16 = np.zeros((S16, D), np.float32)
    xg8 = np.zeros((S8, D), np.float32)
    cs16 = np.zeros(S16, np.float32)
    cs8 = np.zeros(S8, np.float32)
    o16 = o8 = 0
    spans = []  # (e, off16, n16, off8, n8)
    for e in range(E):
        k8 = k8s[e]
        t8, t16 = toks[e][:k8], toks[e][k8:]
        xg8[o8 : o8 + k8] = x[t8]
        cs8[o8 : o8 + k8] = cws[e][:k8] / SC
        xg16[o16 : o16 + len(t16)] = x[t16]
        cs16[o16 : o16 + len(t16)] = cws[e][k8:]
        spans.append((e, o16, len(t16), o8, k8))
        o16 += counts16[e]
        o8 += counts8[e]

    x16d = np.ascontiguousarray(
        xg16.T.reshape(DO, P, S16).transpose(1, 0, 2).astype(BF16)
    )
    x8d = np.ascontiguousarray(
        np.clip(xg8.T / SC, -240, 240)
        .reshape(DO2, 2, P, S8)
        .transpose(2, 0, 1, 3)
        .astype(FP8)
    )

    wkey = (x.ctypes.data, w1.ctypes.data, w2.ctypes.data, w3.ctypes.data)
    if _wcache["key"] != wkey:
        _wcache["key"] = wkey
        _wcache["val"] = _prep_weights(w1, w3, w2)
    wmaps = _wcache["val"]

    in_maps = [
        dict(x16d=x16d, x8d=x8d, cs16d=cs16, cs8d=cs8, **wmaps[i]) for i in range(E)
    ]
    res = run_bass_kernel_spmd(nc, in_maps, core_ids=list(range(E)), trace=_trace)

    O16 = res.results[0]["o16d"].astype(np.float32)
    O8 = res.results[0]["o8d"].astype(np.float32)
    for i in range(1, E):
        O16 += res.results[i]["o16d"].astype(np.float32)
        O8 += res.results[i]["o8d"].astype(np.float32)

    out = np.zeros((T, D), np.float32)
    for e, p16, n16, p8, n8 in spans:
        k8 = k8s[e]
        if n8:
            out[toks[e][:k8]] += O8[:, p8 : p8 + n8].T.astype(np.float32)
        if n16:
            out[toks[e][k8:]] += O16[:, p16 : p16 + n16].T.astype(np.float32)
    if _trace:
        kernel.last_exec_time_ns = res.exec_time_ns
        kernel.last_results = res
    return out



# revision 16
# speedup vs baseline: 1.2373x; 1.2373x over previous
"""MoE feed-forward (top-2 routing, E=8 experts) on 8 TRN2 NeuronCores.

Strategy: 8-way HIDDEN-dim split with host-side routing/dispatch.
  Every core processes ALL experts' gathered tokens, but only a 1/8
  slice of the hidden dimension (4 of 32 ht tiles of 128). Per-core
  work is exactly sum(c_e)/8 token-equivalents -- perfectly balanced
  regardless of routing skew -- and every core runs the SAME static
  program (per-core variation is only in which weight rows/cols the
  host gathers into the same-shaped input arrays).

  Per expert e and token block b (tokens in [feature, token] layout):
    P1: h[4x128, b] = silu((w1 slice) @ x[b]) * ((w3 slice) @ x[b])
    P2: y_partial[D, b] = (w2 slice) @ h  (contraction over the 512
        local h rows).
  The host scales per-token by the routing weight while scatter-adding
  the 8 per-core partial outputs (f32) into [T, D].

Pipeline layout (from trace analysis of the previous version):
  - All queues are in-order; a DMA whose WAR semaphore isn't ready
    head-of-line-blocks everything behind it. So: out-stores ride
    nc.sync alone; x-preloads ride nc.scalar + nc.gpsimd; weights ride
    nc.gpsimd. xt is 5-deep so a preload issued 2 blocks ahead never
    waits on its WAR semaphore at the queue head.
  - pg/pu PSUM tiles are independently double-buffered (4 banks) and
    ps2 4-deep (4 banks): P1 group k+1 never waits on the silu/mul
    readers of group k. PE gaps >3us also drop the PE to a 2x-slower
    p-state, so each avoided gap pays twice.
  - P1 of block k and P2 of block k-1 interleave on the PE at the
    accumulation-group level; the pipeline runs across expert
    boundaries.
"""
import sys

if "/opt/trn_rl_repo" not in sys.path:
    sys.path.insert(0, "/opt/trn_rl_repo")

import numpy as np
import ml_dtypes

import concourse.bass as bass
import concourse.mybir as mybir
from concourse import bacc
from concourse.tile import TileContext
from concourse.bass_utils import run_bass_kernel_spmd

BF16 = ml_dtypes.bfloat16
P = 128
D = 2048     # model dim
H = 4096     # hidden dim
E = 8        # experts
TOP_K = 2
DO = D // P          # 16 contraction tiles for P1
HL = H // P // E     # 4 local ht tiles per core
DT = D // P          # 16 output-row tiles for P2
B16 = 448            # max token block


def _route(x, router_w):
    """Top-2 expert selection + renormalized weights (float64 host math)."""
    logits = x.astype(np.float64) @ router_w.astype(np.float64).T
    m = logits.max(axis=1, keepdims=True)
    p = np.exp(logits - m)
    p /= p.sum(axis=1, keepdims=True)
    sel = np.argsort(-p, axis=1, kind="stable")[:, :TOP_K]
    rw = np.take_along_axis(p, sel, axis=1)
    rw /= rw.sum(axis=1, keepdims=True)
    return sel, rw.astype(np.float32)


def _seg_blocks(L, maxb, first_small=False, last_small=False):
    """Split a 16-multiple segment into 16-multiple blocks <= maxb.
    first_small carves a 256 head (shrinks the startup DMA wall);
    last_small carves a 96 tail (shrinks the final P2 drain)."""
    if L == 0:
        return []
    head = tail = 0
    if first_small and L > 256:
        head, L = 256, L - 256
    if last_small and L > 256:
        tail, L = 96, L - 96
    sizes = []
    if L:
        n = -(-L // maxb)
        base = (L // n) // 16 * 16
        k = (L - base * n) // 16
        sizes = [base + 16] * k + [base] * (n - k)
    if head:
        sizes = [head] + sizes
    if tail:
        sizes = sizes + [tail]
    return sizes


def _plan(counts16):
    """Block plan: list of (expert, seg_off, tn)."""
    blocks = []
    off16 = 0
    for e in range(E):
        t = 0
        # no first_small head: a small head block makes startup P1 groups
        # SHORTER than the HBM feed rate for their weights (1 MB per group
        # at ~2.8us/MB); natural ~344-token groups run ~4.8us and the ramp
        # stays fed.
        for tn in _seg_blocks(counts16[e], B16, last_small=(e == E - 1)):
            blocks.append((e, off16 + t, tn))
            t += tn
        off16 += counts16[e]
    return blocks, max(off16, 16)


def _build(counts16):
    """Bass program: per-core 1/8-H slice of all experts' GLU MLPs."""
    f32 = mybir.dt.float32
    bf16 = mybir.dt.bfloat16
    Silu = mybir.ActivationFunctionType.Silu

    blocks, S16 = _plan(counts16)

    nc = bacc.Bacc(None, target_bir_lowering=False)
    x16d = nc.dram_tensor("x16d", [P, DO, S16], bf16, kind="ExternalInput")
    w1d16 = nc.dram_tensor("w1d16", [P, E, HL, DO, P], bf16, kind="ExternalInput")
    w3d16 = nc.dram_tensor("w3d16", [P, E, HL, DO, P], bf16, kind="ExternalInput")
    w2d16 = nc.dram_tensor("w2d16", [P, E, 4, 4, HL, P], bf16, kind="ExternalInput")
    o16d = nc.dram_tensor("o16d", [D, S16], bf16, kind="ExternalOutput")
    o16v = o16d.rearrange("(dt p) c -> p dt c", p=P)

    with TileContext(nc) as tc:
        with (
            tc.tile_pool(name="x16p", bufs=5) as x16p,
            tc.tile_pool(name="h16p", bufs=3) as h16p,
            tc.tile_pool(name="w1p16", bufs=8) as w1p16,
            tc.tile_pool(name="w3p16", bufs=8) as w3p16,
            tc.tile_pool(name="w2p16", bufs=8) as w2p16,
            tc.tile_pool(name="stp", bufs=3) as stp,
            tc.tile_pool(name="yp", bufs=6) as yp,
            tc.tile_pool(name="ps13", bufs=2, space="PSUM") as ps13,
            tc.tile_pool(name="ps2", bufs=4, space="PSUM") as ps2,
        ):
            w1t16 = [[None] * HL for _ in range(E)]
            w3t16 = [[None] * HL for _ in range(E)]
            w2t16 = [[None] * 4 for _ in range(E)]

            def load_w13(e, hl, q1=None, q3=None):
                w1t16[e][hl] = w1p16.tile([P, DO, P], bf16, name="w1t16")
                (q1 or nc.gpsimd).dma_start(out=w1t16[e][hl][:], in_=w1d16[:, e, hl])
                w3t16[e][hl] = w3p16.tile([P, DO, P], bf16, name="w3t16")
                (q3 or nc.gpsimd).dma_start(out=w3t16[e][hl][:], in_=w3d16[:, e, hl])

            def load_w2(e, g, q=None):
                w2t16[e][g] = w2p16.tile([P, 4, HL, P], bf16, name="w2t16")
                (q or nc.gpsimd).dma_start(out=w2t16[e][g][:], in_=w2d16[:, e, g])

            def wload_list(e):
                """Closures emitting expert e's weight DMAs, in need order."""
                ops = []
                for hl in range(HL):
                    ops.append(lambda e=e, hl=hl: load_w13(e, hl))
                for g in range(4):
                    ops.append(lambda e=e, g=g: load_w2(e, g))
                return ops

            def load_block(blk, xt):
                """x rides the SAME gpsimd ring as the weight stream, posted
                at block start = ahead of that block's weight closures, so
                the single FIFO serves transfers in exact need order. A
                different ring would starve: SDMA engines drain a whole
                packet per ring, and the SWDGE weight ring wins ~10:1 over
                an HWDGE ring whenever it has backlog (measured: 0.44MB x
                taking 13-18us on sync during weight bursts). Never on
                scalar either: a DMA ahead of a silu head-of-line-blocks
                it. xt is 5 blocks deep so the WAR semaphore is satisfied
                at issue."""
                e, o, tn = blk
                h = DO // 2
                nc.gpsimd.dma_start(out=xt[:, :h, :tn], in_=x16d[:, :h, o : o + tn])
                nc.gpsimd.dma_start(out=xt[:, h:, :tn], in_=x16d[:, h:, o : o + tn])

            def p1_group(blk, xt, ht, hl):
                e, o, tn = blk
                pg = ps13.tile([P, 512], f32, name="pg", tag="pg")[:, :tn]
                pu = ps13.tile([P, 512], f32, name="pu", tag="pu")[:, :tn]
                for dk in range(DO):
                    nc.tensor.matmul(
                        pg, w1t16[e][hl][:, dk], xt[:, dk, :tn],
                        start=(dk == 0), stop=(dk == DO - 1),
                    )
                    nc.tensor.matmul(
                        pu, w3t16[e][hl][:, dk], xt[:, dk, :tn],
                        start=(dk == 0), stop=(dk == DO - 1),
                    )
                st = stp.tile([P, 512], f32, name="st")[:, :tn]
                nc.scalar.activation(st, pg, Silu)
                nc.vector.tensor_mul(ht[:, hl, :tn], st, pu)

            def p2_group(blk, ht, dt, drain=False):
                e, o, tn = blk
                # final drain: P1 is done, so its 4 PSUM banks are free --
                # alternate pools for an 8-deep rotation so the PE never
                # waits on the copy/store chain.
                if drain and dt % 2 == 1:
                    py = ps13.tile([P, 512], f32, name="py2", tag="pg" if dt % 4 == 1 else "pu")[:, :tn]
                else:
                    py = ps2.tile([P, 512], f32, name="py")[:, :tn]
                for hl in range(HL):
                    nc.tensor.matmul(
                        py, w2t16[e][dt // 4][:, dt % 4, hl], ht[:, hl, :tn],
                        start=(hl == 0), stop=(hl == HL - 1),
                    )
                ysb = yp.tile([P, 512], bf16, name="ysb")[:, :tn]
                nc.vector.tensor_copy(ysb, py)
                # final expert: split out-DMAs with the (by now idle) weight
                # queue so the drain tail isn't serialized on one queue
                if e == E - 1:
                    q = (nc.sync, nc.gpsimd, nc.scalar)[dt % 3]
                else:
                    q = nc.sync
                q.dma_start(out=o16v[:, dt, o : o + tn], in_=ysb)

            # ---- emission: startup spreads first loads over 3 queues ----
            bq = []  # xt tiles loaded, awaiting compute

            def preload(j):
                blk = blocks[j]
                xt = x16p.tile([P, DO, B16], bf16, name="xt16")
                load_block(blk, xt)
                bq.append((blk, xt))

            # first P1 group needs w13[0][0] + x block0: put them on
            # separate fast queues so the first MM fires ~4us in.
            load_w13(0, 0, q1=nc.sync, q3=nc.scalar)
            preload(0)
            load_w13(0, 1)
            load_w13(0, 2)
            load_w13(0, 3)
            preload(1)
            # expert-0 w2 is only needed when block 1 starts (P2 of block 0
            # interleaves with P1 of block 1); defer it into the throttled
            # stream so the startup window stays under the HBM ceiling.
            pending = [lambda g=g: load_w2(0, g) for g in range(4)]
            pending += wload_list(1)
            next_loaded = 1

            prev = None  # (blk, ht) awaiting its P2
            for k, blk in enumerate(blocks):
                e, o, tn = blk
                if k + 2 < len(blocks):
                    preload(k + 2)
                while next_loaded < min(e + 2, E):
                    pending += wload_list(next_loaded)
                    next_loaded += 1
                _, xt = bq.pop(0)
                ht = h16p.tile([P, HL, B16], bf16, name="ht16")
                for hl in range(HL):
                    p1_group(blk, xt, ht, hl)
                    if prev is not None:
                        for dt in range(4 * hl, 4 * hl + 4):
                            p2_group(*prev, dt)
                    # throttle the weight stream: HBM is ~358 GB/s shared
                    # with x-preloads and out-stores, and an over-eager
                    # prefetch starves the x stream during the short
                    # startup blocks (a w13 closure is 1 MB). 1 closure
                    # per P1 group = 4/block vs 8 closures per expert per
                    # >=1 block: stays >=1 expert ahead at ~1/4 the burst
                    # bandwidth.
                    for f in pending[:1]:
                        f()
                    pending = pending[1:]
                prev = (blk, ht)
            for f in pending:
                f()
            for dt in range(DT):
                p2_group(*prev, dt, drain=True)

    nc.compile()
    return nc, blocks, S16


_cache = {}


def _get_program(counts16):
    key = tuple(counts16)
    if key not in _cache:
        _cache[key] = _build(counts16)
    return _cache[key]


def _prep_weights(w1, w3, w2):
    """Per-core weight arrays in device layouts (see _build docstring)."""
    per_core = []
    w1_16 = w1.astype(BF16)
    w3_16 = w3.astype(BF16)
    w2_16 = w2.astype(BF16)
    for i in range(E):
        r0 = 512 * i
        W116 = np.empty((P, E, HL, DO, P), BF16)
        W316 = np.empty((P, E, HL, DO, P), BF16)
        W216 = np.empty((P, E, 4, 4, HL, P), BF16)
        for e in range(E):
            W116[:, e] = w1_16[e, r0 : r0 + 512].reshape(HL, P, DO, P).transpose(3, 0, 2, 1)
            W316[:, e] = w3_16[e, r0 : r0 + 512].reshape(HL, P, DO, P).transpose(3, 0, 2, 1)
            W216[:, e] = (
                w2_16[e, :, r0 : r0 + 512].reshape(4, 4, P, HL, P).transpose(4, 0, 1, 3, 2)
            )
        per_core.append(dict(w1d16=W116, w3d16=W316, w2d16=W216))
    return per_core


_wcache = {"key": None, "val": None}


def kernel(x, router_w, w1, w3, w2, _trace=False):
    T = x.shape[0]
    x = np.asarray(x, np.float32)
    router_w = np.asarray(router_w, np.float32)
    w1 = np.asarray(w1, np.float32)
    w3 = np.asarray(w3, np.float32)
    w2 = np.asarray(w2, np.float32)
    assert x.shape[1] == D and router_w.shape == (E, D)
    assert w1.shape == w3.shape == (E, H, D) and w2.shape == (E, D, H)

    sel, rw = _route(x, router_w)

    # per-expert token lists
    toks, cws = [], []
    for e in range(E):
        mask = sel == e
        tok = np.nonzero(mask.any(axis=1))[0]
        cw = np.where(mask[tok, 0], rw[tok, 0], rw[tok, 1])
        toks.append(tok)
        cws.append(cw)

    counts16 = [-(-len(t) // 16) * 16 if len(t) else 0 for t in toks]
    nc, blocks, S16 = _get_program(counts16)

    # ---- host-side gathers into device layouts ----
    xg16 = np.zeros((S16, D), np.float32)
    o16 = 0
    spans = []  # (e, off16, n16)
    for e in range(E):
        t16 = toks[e]
        xg16[o16 : o16 + len(t16)] = x[t16]
        spans.append((e, o16, len(t16)))
        o16 += counts16[e]

    x16d = np.ascontiguousarray(
        xg16.T.reshape(DO, P, S16).transpose(1, 0, 2).astype(BF16)
    )

    wkey = (x.ctypes.data, w1.ctypes.data, w2.ctypes.data, w3.ctypes.data)
    if _wcache["key"] != wkey:
        _wcache["key"] = wkey
        _wcache["val"] = _prep_weights(w1, w3, w2)
    wmaps = _wcache["val"]

    in_maps = [dict(x16d=x16d, **wmaps[i]) for i in range(E)]
    res = run_bass_kernel_spmd(nc, in_maps, core_ids=list(range(E)), trace=_trace)

    O16 = res.results[0]["o16d"].astype(np.float32)
    for i in range(1, E):
        O16 += res.results[i]["o16d"].astype(np.float32)

    out = np.zeros((T, D), np.float32)
    for e, p16, n16 in spans:
        if n16:
            out[toks[e]] += cws[e][:, None] * O16[:, p16 : p16 + n16].T
    if _trace:
        kernel.last_exec_time_ns = res.exec_time_ns
        kernel.last_results = res
    return out
